# revision 5
# baseline (speedup 1.0000x reference)
"""MultiHeadAttention Trainium2 kernel (8 NeuronCores, SPMD).

Sharding: core c = (batch b=c//2, head-group g=c%2). Each core computes
8 of the 16 heads for one batch: Q/K/V projections restricted to the
512 d_model dims of its head group, full attention for those heads, and
a partial output projection. Host sums the two head-group partials per
batch and adds the output bias.

Single merged pipeline (v2). The kernel is ACT(exp)-floor-limited at
~278us (256 x [128,1024] exps at 1.0855us each) and PE-streaming-limited
at ~330us (1536 N=512 matmuls). All phases overlap:
  - DMA order: biases, wq, xq, wk(pair0), xk, wv, xv, wk(pairs1-3), wo.
    Exps can start as soon as qhT/khT pair0 exist (~44us, DMA-bound).
  - ALL Q projections run in the DMA-bound preamble (PE idle anyway).
  - scores+exp are emitted with tc.high_priority: the Tile scheduler
    always prefers them, so ACT never waits on PE for its next tile.
  - K pairs 1-3, V projection, ctx matmuls, and the output projection
    fill remaining PE slots by dependency readiness (priority heap).
  - chunks run qh-major so the output projection for query-half 0 can
    overlap the entire second half of attention.
  - xv reuses the xq SBUF tiles (WAR after Q proj, which is done by the
    time xk finishes); wo/wk134 arrive during attention.

PSUM: sps 2x[128,1024] (4 banks) + ctx accum [128,1024] (2) + shared
proj/outproj pool 2x[128,512] (2) = 8 banks exactly.
"""

import numpy as np
import ml_dtypes

BF16 = ml_dtypes.bfloat16

B, S, D, H = 4, 2048, 1024, 16
DH = 64          # head dim
DL = 512         # local d_out (8 heads)
P = 128          # partitions
NCORES = 8
SCALE = 1.0 / np.sqrt(DH)
HIP = 1_000_000  # high-priority offset for scores+exp

_CACHE = {}
LAST_RESULTS = None  # stashed BassKernelResults for test harness


def _build_nc():
    import concourse.bass as bass
    from concourse import bacc, mybir
    from concourse.tile import TileContext

    f32 = mybir.dt.float32
    bf16 = mybir.dt.bfloat16

    nc = bacc.Bacc("TRN2", target_bir_lowering=False, debug=False, num_devices=NCORES)

    # x pre-transposed on host: (8 d_in chunks, 128, 2048 tokens)
    xq_d = nc.dram_tensor("xq", (8, P, S), bf16, kind="ExternalInput")
    xk_d = nc.dram_tensor("xk", (8, P, S), bf16, kind="ExternalInput")
    xv_d = nc.dram_tensor("xv", (8, P, S), bf16, kind="ExternalInput")
    # wq/wk pair-major: (P, 4 pairs, 8 c-chunks, 128) so a pair is one
    # contiguous 2KB/partition DMA
    wq_d = nc.dram_tensor("wq", (P, 4, 8, P), bf16, kind="ExternalInput")
    wk_d = nc.dram_tensor("wk", (P, 4, 8, P), bf16, kind="ExternalInput")
    wv_d = nc.dram_tensor("wv", (P, 8, DL), bf16, kind="ExternalInput")
    wo_d = nc.dram_tensor("wo", (P, 4, D), bf16, kind="ExternalInput")
    bq_d = nc.dram_tensor("bq", (P, 4), f32, kind="ExternalInput")
    bk_d = nc.dram_tensor("bk", (P, 4), f32, kind="ExternalInput")
    bvb_d = nc.dram_tensor("bvb", (P, 520), bf16, kind="ExternalInput")
    out_d = nc.dram_tensor("out", (S, D), bf16, kind="ExternalOutput")

    Exp = mybir.ActivationFunctionType.Exp

    with TileContext(nc) as tc:
        with tc.tile_pool(name="res", bufs=1) as res, \
             tc.tile_pool(name="pp", bufs=2, space="PSUM") as pp, \
             tc.tile_pool(name="sc", bufs=2, space="PSUM") as sc, \
             tc.tile_pool(name="cx", bufs=1, space="PSUM") as cx, \
             tc.tile_pool(name="pt", bufs=11) as ptp, \
             tc.tile_pool(name="st", bufs=1) as stp, \
             tc.tile_pool(name="sm", bufs=1) as sm, \
             tc.tile_pool(name="ot", bufs=2) as otp:

            # ---------------- resident SBUF ----------------
            wq_sb = res.tile([P, 4, 8, P], bf16)
            wk_sb = res.tile([P, 4, 8, P], bf16)
            wv_sb = res.tile([P, 8, DL], bf16)
            wo_sb = res.tile([P, 4, D], bf16)
            bq_sb = res.tile([P, 4], f32)
            bk_sb = res.tile([P, 4], f32)
            bvb_sb = res.tile([P, 520], bf16)

            xa = [res.tile([P, S], bf16, name=f"xa{c}") for c in range(8)]
            xb = [res.tile([P, S], bf16, name=f"xb{c}") for c in range(8)]

            qhT = [res.tile([P, S], bf16, name=f"qhT{i}") for i in range(4)]
            khT = [res.tile([P, S], bf16, name=f"khT{i}") for i in range(4)]
            vh = [res.tile([P, 8 * 65], bf16, name=f"vh{i}") for i in range(16)]
            ctxT = [res.tile([P, 1024], bf16, name=f"ctxT{i}") for i in range(8)]

            # ---------------- DMAs in priority order ----------------
            # scalar (ACT) queue is free until exps start (~44us): use it
            # for xq/xk; xv/wk123/wo overlap the exp phase -> sync/gpsimd
            qs = [nc.sync, nc.scalar, nc.gpsimd]
            q2 = [nc.sync, nc.gpsimd]
            nc.sync.dma_start(out=bq_sb, in_=bq_d.ap())
            nc.scalar.dma_start(out=bk_sb, in_=bk_d.ap())
            nc.gpsimd.dma_start(out=bvb_sb, in_=bvb_d.ap())
            nc.sync.dma_start(out=wq_sb, in_=wq_d.ap())
            for c in range(8):
                qs[c % 3].dma_start(out=xa[c], in_=xq_d.ap()[c])
            nc.sync.dma_start(out=wk_sb[:, 0], in_=wk_d.ap()[:, 0])
            for c in range(8):
                qs[c % 3].dma_start(out=xb[c], in_=xk_d.ap()[c])
            nc.sync.dma_start(out=wv_sb, in_=wv_d.ap())

            # ---------------- projections ----------------
            def proj_qk(x_sb, w_sb, b_sb, dst, pairs):
                # x-streaming, one PSUM tile at a time (c-inner): only one
                # accumulation group live -> fits the 2-slot pp pool
                for p in pairs:
                    for t in range(4):
                        ps = pp.tile([P, 512], f32, name="pjt", tag="pp")
                        for c in range(8):
                            nc.tensor.matmul(
                                ps, lhsT=w_sb[:, p, c, :],
                                rhs=x_sb[c][:, t * 512:(t + 1) * 512],
                                start=(c == 0), stop=(c == 7))
                        nc.vector.tensor_scalar_add(
                            out=dst[p][:, t * 512:(t + 1) * 512],
                            in0=ps, scalar1=b_sb[:, p:p + 1])

            # all Q pairs fit in the DMA-bound preamble; K0 gates exp start
            proj_qk(xa, wq_sb, bq_sb, qhT, range(4))
            # xv reuses the xa tiles: emitted AFTER the Q projection so each
            # chunk's DMA waits (WAR) for Q-proj's reads (Tile deps follow
            # emission order)
            for c in range(8):
                q2[c % 2].dma_start(out=xa[c], in_=xv_d.ap()[c])
            nc.sync.dma_start(out=wk_sb[:, 1:4], in_=wk_d.ap()[:, 1:4])
            nc.sync.dma_start(out=wo_sb, in_=wo_d.ap())
            proj_qk(xb, wk_sb, bk_sb, khT, [0])

            # V projection: x-stationary so the output lands token-major.
            # Emitted before the attention chunks (and before K1-3) so the
            # scheduler prefers it as soon as xv lands -> vh[kb] feeds ctx.
            for tb in range(16):
                psv = pp.tile([P, 512], f32, name="psv", tag="pp")
                for c in range(8):
                    nc.tensor.matmul(
                        psv, lhsT=xa[c][:, tb * P:(tb + 1) * P],
                        rhs=wv_sb[:, c, :],
                        start=(c == 0), stop=(c == 7))
                vt = vh[tb].rearrange("p (h e) -> p h e", e=65)
                nc.vector.tensor_copy(
                    vt[:, :, 0:64],
                    psv.rearrange("p (h e) -> p h e", e=64))
                nc.gpsimd.memset(vt[:, :, 64:65], 1.0)
                nc.vector.tensor_add(vh[tb], vh[tb], bvb_sb)

            proj_qk(xb, wk_sb, bk_sb, khT, [1, 2, 3])

            # ---------------- attention ----------------
            def attn_chunk(p, hh, qh):
                h = 2 * p + hh
                po = 64 * hh
                q0 = qh * 1024
                cps = cx.tile([P, 1024], f32, name="cps", tag="cx")
                pts = []
                for kb in range(16):
                    with tc.high_priority(offset=HIP):
                        sps = sc.tile([P, 1024], f32, name="sps", tag="s")
                        for j in range(2):
                            nc.tensor.matmul(
                                sps[:, j * 512:(j + 1) * 512],
                                lhsT=khT[p][po:po + 64, kb * P:(kb + 1) * P],
                                rhs=qhT[p][po:po + 64, q0 + j * 512:q0 + (j + 1) * 512],
                                start=True, stop=True)
                        pt = ptp.tile([P, 1024], bf16, name="ptt", tag="pt")
                        nc.scalar.activation(pt, sps, Exp, scale=SCALE)
                    for j in range(2):
                        nc.tensor.matmul(
                            cps[0:65, j * 512:(j + 1) * 512],
                            lhsT=vh[kb][:, 65 * h:65 * h + 65],
                            rhs=pt[:, j * 512:(j + 1) * 512],
                            start=(kb == 0), stop=(kb == 15))
                # stage ctx+denominator to SBUF, freeing the PSUM bank;
                # normalize on DVE off the PE critical path
                stg = stp.tile([P, 1024], f32, name="stg", tag="st")
                nc.vector.tensor_copy(stg[0:65, :], cps[0:65, :])
                rc = sm.tile([1, 1024], f32, name="rc", tag="rc")
                nc.gpsimd.dma_start(out=rc, in_=stg[64:65, :])
                step = (list(rc.ap[1])[0]
                        if hasattr(rc.ap[1], "__iter__") else 1)
                bc = sm.tile([64, 1024], f32, name="bc", tag="bc")
                nc.gpsimd.dma_start(
                    out=bc,
                    in_=bass.AP(tensor=rc.tensor, offset=rc.offset,
                                ap=[[1, 1], [0, 64], [step, 1024]]))
                rb = sm.tile([64, 1024], f32, name="rb", tag="rb")
                nc.vector.reciprocal_approx_fast(rb, bc)
                nc.vector.tensor_mul(
                    ctxT[2 * p + qh][po:po + 64, :], stg[0:64, :], rb)

            # ---------------- output projection ----------------
            def outproj(qh):
                for qbl in range(8):
                    qb = qh * 8 + qbl
                    for n in range(2):
                        oa = pp.tile([P, 512], f32, name="ops", tag="pp")
                        for p in range(4):
                            nc.tensor.matmul(
                                oa,
                                lhsT=ctxT[2 * p + qh][:, qbl * P:(qbl + 1) * P],
                                rhs=wo_sb[:, p, n * 512:(n + 1) * 512],
                                start=(p == 0), stop=(p == 3))
                        ot = otp.tile([P, 512], bf16, name="ot", tag="ot")
                        nc.vector.tensor_copy(ot, oa)
                        [nc.sync, nc.gpsimd][qb % 2].dma_start(
                            out=out_d.ap()[qb * P:(qb + 1) * P,
                                           n * 512:(n + 1) * 512],
                            in_=ot)

            # qh-major: all of query-half 0 first, so its output projection
            # overlaps the second half of attention
            for qh in range(2):
                for p in range(4):
                    for hh in range(2):
                        attn_chunk(p, hh, qh)
                outproj(qh)

    nc.finalize()
    return nc


def _prep_in_maps(q, k, v, Wq, bq, Wk, bk, Wv, bv, Wo, bo):
    in_maps = []
    for c in range(NCORES):
        b, g = c // 2, c % 2
        sl = slice(g * DL, (g + 1) * DL)
        bvl = np.asarray(bv)[sl].astype(np.float32)
        bvb = np.zeros(520, np.float32)
        for h in range(8):
            bvb[65 * h:65 * h + 64] = bvl[64 * h:64 * h + 64]
        bvb = np.broadcast_to(bvb, (P, 520))
        def tile_x(x):
            xt = np.ascontiguousarray(np.asarray(x)[b].T).astype(BF16)  # (1024, 2048)
            return xt.reshape(8, P, S)
        def tile_w_pair(w):
            # (1024, 512) d_in x d_out_local -> (P, 4 pairs, 8 c, 128)
            return np.ascontiguousarray(
                np.asarray(w).reshape(8, P, 4, P).transpose(1, 2, 0, 3))
        def tile_w(w):
            # (1024, DL) -> (P, 8, DL) partition-major
            return np.ascontiguousarray(
                np.asarray(w).reshape(8, P, -1).transpose(1, 0, 2))
        in_maps.append({
            "xq": tile_x(q),
            "xk": tile_x(k),
            "xv": tile_x(v),
            "wq": tile_w_pair(np.asarray(Wq)[sl, :].T.astype(BF16)),
            "wk": tile_w_pair(np.asarray(Wk)[sl, :].T.astype(BF16)),
            "wv": tile_w(np.asarray(Wv)[sl, :].T.astype(BF16)),
            "wo": np.ascontiguousarray(
                np.asarray(Wo)[:, sl].T.astype(BF16).reshape(4, P, D).transpose(1, 0, 2)),
            "bq": np.ascontiguousarray(np.asarray(bq)[sl].reshape(4, P).T).astype(np.float32),
            "bk": np.ascontiguousarray(np.asarray(bk)[sl].reshape(4, P).T).astype(np.float32),
            "bvb": np.ascontiguousarray(bvb).astype(BF16),
        })
    return in_maps


def _get_runner():
    """Build nc + jitted SPMD executor once; reuse across kernel() calls."""
    if "runner" in _CACHE:
        return _CACHE["runner"]
    import jax
    import jax.numpy as jnp
    from jax.sharding import Mesh, PartitionSpec
    from jax.experimental.shard_map import shard_map
    from concourse import mybir
    from concourse.bass2jax import (_bass_exec_p, install_neuronx_cc_hook,
                                    partition_id_tensor)

    nc = _build_nc()
    install_neuronx_cc_hook()

    partition_name = nc.partition_id_tensor.name if nc.partition_id_tensor else None
    in_names, out_names, out_avals, zero_shapes = [], [], [], []
    for alloc in nc.m.functions[0].allocations:
        if not isinstance(alloc, mybir.MemoryLocationSet):
            continue
        name = alloc.memorylocations[0].name
        if alloc.kind == "ExternalInput":
            if name != partition_name:
                in_names.append(name)
        elif alloc.kind == "ExternalOutput":
            shape = tuple(alloc.tensor_shape)
            dtype = mybir.dt.np(alloc.dtype)
            out_names.append(name)
            out_avals.append(jax.core.ShapedArray(shape, dtype))
            zero_shapes.append((shape, dtype))
    n_params = len(in_names)
    all_in_names = list(in_names) + list(out_names)
    if partition_name is not None:
        all_in_names.append(partition_name)

    def _body(*args):
        operands = list(args)
        if partition_name is not None:
            operands.append(partition_id_tensor())
        outs = _bass_exec_p.bind(
            *operands,
            out_avals=tuple(out_avals),
            in_names=tuple(all_in_names),
            out_names=tuple(out_names),
            lowering_input_output_aliases=(),
            sim_require_finite=True,
            sim_require_nnan=True,
            nc=nc,
        )
        return tuple(outs)

    devices = jax.devices()[:NCORES]
    mesh = Mesh(np.asarray(devices), ("core",))
    n_outs = len(out_names)
    sharded = jax.jit(
        shard_map(_body, mesh=mesh,
                  in_specs=(PartitionSpec("core"),) * (n_params + n_outs),
                  out_specs=(PartitionSpec("core"),) * n_outs,
                  check_rep=False),
        donate_argnums=tuple(range(n_params, n_params + n_outs)),
        keep_unused=True,
    )
    runner = dict(nc=nc, sharded=sharded, in_names=in_names,
                  out_names=out_names, zero_shapes=zero_shapes,
                  out_avals=out_avals)
    _CACHE["runner"] = runner
    return runner


def kernel(q, k, v, Wq, bq, Wk, bk, Wv, bv, Wo, bo):
    global LAST_RESULTS
    r = _get_runner()
    in_maps = _prep_in_maps(q, k, v, Wq, bq, Wk, bk, Wv, bv, Wo, bo)

    concat_in = [np.concatenate([m[name] for m in in_maps], axis=0)
                 for name in r["in_names"]]
    concat_zeros = [np.zeros((NCORES * s[0], *s[1:]), d)
                    for (s, d) in r["zero_shapes"]]
    out_arrs = r["sharded"](*concat_in, *concat_zeros)
    results = [
        {name: np.asarray(out_arrs[i]).reshape(NCORES, *r["out_avals"][i].shape)[c]
         for i, name in enumerate(r["out_names"])}
        for c in range(NCORES)
    ]
    LAST_RESULTS = results

    bo_f = np.asarray(bo).astype(np.float32)
    out = np.empty((B, S, D), np.float32)
    for b in range(B):
        out[b] = (results[2 * b]["out"].astype(np.float32)
                  + results[2 * b + 1]["out"].astype(np.float32)
                  + bo_f)
    return out


# revision 14
# speedup vs baseline: 1.0199x; 1.0199x over previous
"""MultiHeadAttention Trainium2 kernel (8 NeuronCores, SPMD).

Sharding: core c = (batch b=c//2, head-group g=c%2). Each core computes
8 of the 16 heads for one batch: Q/K/V projections restricted to the
512 d_model dims of its head group, full attention for those heads, and
a partial output projection. Host sums the two head-group partials per
batch and adds the output bias.

Single merged pipeline (v2). The kernel is ACT(exp)-floor-limited at
~278us (256 x [128,1024] exps at 1.0855us each) and PE-streaming-limited
at ~330us (1536 N=512 matmuls). All phases overlap:
  - DMA order: biases, wq, xq, wk(pair0), xk, wv, xv, wk(pairs1-3), wo.
    Exps can start as soon as qhT/khT pair0 exist (~44us, DMA-bound).
  - ALL Q projections run in the DMA-bound preamble (PE idle anyway).
  - scores+exp are emitted with tc.high_priority: the Tile scheduler
    always prefers them, so ACT never waits on PE for its next tile.
  - K pairs 1-3, V projection, ctx matmuls, and the output projection
    fill remaining PE slots by dependency readiness (priority heap).
  - chunks run qh-major so the output projection for query-half 0 can
    overlap the entire second half of attention.
  - xv reuses the xq SBUF tiles (WAR after Q proj, which is done by the
    time xk finishes); wo/wk134 arrive during attention.

PSUM: sps 2x[128,1024] (4 banks) + ctx accum [128,1024] (2) + shared
proj/outproj pool 2x[128,512] (2) = 8 banks exactly.
"""

import numpy as np
import ml_dtypes

BF16 = ml_dtypes.bfloat16

B, S, D, H = 4, 2048, 1024, 16
DH = 64          # head dim
DL = 512         # local d_out (8 heads)
P = 128          # partitions
NCORES = 8
SCALE = 1.0 / np.sqrt(DH)
HIP = 1_000_000  # high-priority offset for scores+exp

_CACHE = {}
LAST_RESULTS = None  # stashed BassKernelResults for test harness


def _build_nc():
    import concourse.bass as bass
    from concourse import bacc, mybir
    from concourse.tile import TileContext

    f32 = mybir.dt.float32
    bf16 = mybir.dt.bfloat16

    nc = bacc.Bacc("TRN2", target_bir_lowering=False, debug=False, num_devices=NCORES)

    # x pre-transposed on host: (8 d_in chunks, 128, 2048 tokens)
    xq_d = nc.dram_tensor("xq", (8, P, S), bf16, kind="ExternalInput")
    xk_d = nc.dram_tensor("xk", (8, P, S), bf16, kind="ExternalInput")
    xv_d = nc.dram_tensor("xv", (8, P, S), bf16, kind="ExternalInput")
    # wq/wk pair-major: (P, 4 pairs, 8 c-chunks, 128) so a pair is one
    # contiguous 2KB/partition DMA
    wq_d = nc.dram_tensor("wq", (P, 4, 8, P), bf16, kind="ExternalInput")
    wk0_d = nc.dram_tensor("wk0", (P, 8, P), bf16, kind="ExternalInput")
    wk123_d = nc.dram_tensor("wk123", (P, 3, 8, P), bf16, kind="ExternalInput")
    wv_d = nc.dram_tensor("wv", (P, 8, DL), bf16, kind="ExternalInput")
    wo_d = nc.dram_tensor("wo", (P, 4, D), bf16, kind="ExternalInput")
    bq_d = nc.dram_tensor("bq", (P, 4), f32, kind="ExternalInput")
    bk_d = nc.dram_tensor("bk", (P, 4), f32, kind="ExternalInput")
    bvb_d = nc.dram_tensor("bvb", (P, 520), bf16, kind="ExternalInput")
    out_d = nc.dram_tensor("out", (S, D), bf16, kind="ExternalOutput")

    Exp = mybir.ActivationFunctionType.Exp

    with TileContext(nc) as tc:
        with tc.tile_pool(name="res", bufs=1) as res, \
             tc.tile_pool(name="pp", bufs=2, space="PSUM") as pp, \
             tc.tile_pool(name="sc", bufs=2, space="PSUM") as sc, \
             tc.tile_pool(name="cx", bufs=1, space="PSUM") as cx, \
             tc.tile_pool(name="pt", bufs=11) as ptp, \
             tc.tile_pool(name="st", bufs=2) as stp, \
             tc.tile_pool(name="sm", bufs=2) as sm, \
             tc.tile_pool(name="ot", bufs=2) as otp:

            # ---------------- resident SBUF ----------------
            wq_sb = res.tile([P, 4, 8, P], bf16)
            wk0_sb = res.tile([P, 8, P], bf16)
            wk123_sb = res.tile([P, 3, 8, P], bf16)
            wv_sb = res.tile([P, 8, DL], bf16)
            wo_sb = res.tile([P, 4, D], bf16)
            bq_sb = res.tile([P, 4], f32)
            bk_sb = res.tile([P, 4], f32)
            bvb_sb = res.tile([P, 520], bf16)

            xa = [res.tile([P, S], bf16, name=f"xa{c}") for c in range(8)]
            xb = [res.tile([P, S], bf16, name=f"xb{c}") for c in range(8)]

            qhT = [res.tile([P, S], bf16, name=f"qhT{i}") for i in range(4)]
            khT = [res.tile([P, S], bf16, name=f"khT{i}") for i in range(4)]
            vh = [res.tile([P, 8 * 65], bf16, name=f"vh{i}") for i in range(16)]
            ctxT = [res.tile([P, 1024], bf16, name=f"ctxT{i}") for i in range(8)]

            # ---------------- DMAs in priority order ----------------
            # scalar (ACT) queue is free until exps start (~44us): use it
            # for xq/xk; xv/wk123/wo overlap the exp phase -> sync/gpsimd
            qs = [nc.sync, nc.scalar, nc.gpsimd]
            q2 = [nc.sync, nc.gpsimd]
            nc.sync.dma_start(out=bq_sb, in_=bq_d.ap())
            nc.scalar.dma_start(out=bk_sb, in_=bk_d.ap())
            nc.gpsimd.dma_start(out=bvb_sb, in_=bvb_d.ap())
            nc.sync.dma_start(out=wq_sb, in_=wq_d.ap())
            for c in range(8):
                qs[c % 3].dma_start(out=xa[c], in_=xq_d.ap()[c])
            nc.sync.dma_start(out=wk0_sb, in_=wk0_d.ap())
            for c in range(8):
                qs[c % 3].dma_start(out=xb[c], in_=xk_d.ap()[c])
            nc.sync.dma_start(out=wv_sb, in_=wv_d.ap())

            # ---------------- projections ----------------
            def proj_qk(x_sb, wsel, b_sb, dst, pairs):
                # x-streaming, one PSUM tile at a time (c-inner): only one
                # accumulation group live -> fits the 2-slot pp pool
                for p in pairs:
                    for t in range(4):
                        ps = pp.tile([P, 512], f32, name="pjt", tag="pp")
                        for c in range(8):
                            nc.tensor.matmul(
                                ps, lhsT=wsel(p, c),
                                rhs=x_sb[c][:, t * 512:(t + 1) * 512],
                                start=(c == 0), stop=(c == 7))
                        nc.vector.tensor_scalar_add(
                            out=dst[p][:, t * 512:(t + 1) * 512],
                            in0=ps, scalar1=b_sb[:, p:p + 1])

            wq_w = lambda p, c: wq_sb[:, p, c, :]
            wk_w = lambda p, c: (wk0_sb[:, c, :] if p == 0
                                 else wk123_sb[:, p - 1, c, :])
            # all Q pairs fit in the DMA-bound preamble; K0 gates exp start
            proj_qk(xa, wq_w, bq_sb, qhT, range(4))
            # xv reuses the xa tiles: emitted AFTER the Q projection so each
            # chunk's DMA waits (WAR) for Q-proj's reads (Tile deps follow
            # emission order)
            for c in range(8):
                q2[c % 2].dma_start(out=xa[c], in_=xv_d.ap()[c])
            nc.sync.dma_start(out=wk123_sb, in_=wk123_d.ap())
            nc.sync.dma_start(out=wo_sb, in_=wo_d.ap())
            proj_qk(xb, wk_w, bk_sb, khT, [0])
            # K1 fills the PE while xv is still in flight (43-58us) and
            # must be done before chunk 2 (pair 1) needs it
            proj_qk(xb, wk_w, bk_sb, khT, [1])

            # V projection: x-stationary so the output lands token-major.
            # Emitted before the attention chunks (and before K1-3) so the
            # scheduler prefers it as soon as xv lands -> vh[kb] feeds ctx.
            for tb in range(16):
                psv = pp.tile([P, 512], f32, name="psv", tag="pp")
                for c in range(8):
                    nc.tensor.matmul(
                        psv, lhsT=xa[c][:, tb * P:(tb + 1) * P],
                        rhs=wv_sb[:, c, :],
                        start=(c == 0), stop=(c == 7))
                vt = vh[tb].rearrange("p (h e) -> p h e", e=65)
                nc.vector.tensor_copy(
                    vt[:, :, 0:64],
                    psv.rearrange("p (h e) -> p h e", e=64))
                nc.gpsimd.memset(vt[:, :, 64:65], 1.0)
                nc.vector.tensor_add(vh[tb], vh[tb], bvb_sb)

            proj_qk(xb, wk_w, bk_sb, khT, [2, 3])

            # ---------------- attention ----------------
            def attn_chunk(p, hh, qh):
                h = 2 * p + hh
                po = 64 * hh
                q0 = qh * 1024
                cps = cx.tile([P, 1024], f32, name="cps", tag="cx")
                pts = []
                for kb in range(16):
                    with tc.high_priority(offset=HIP):
                        sps = sc.tile([P, 1024], f32, name="sps", tag="s")
                        for j in range(2):
                            nc.tensor.matmul(
                                sps[:, j * 512:(j + 1) * 512],
                                lhsT=khT[p][po:po + 64, kb * P:(kb + 1) * P],
                                rhs=qhT[p][po:po + 64, q0 + j * 512:q0 + (j + 1) * 512],
                                start=True, stop=True)
                        pt = ptp.tile([P, 1024], bf16, name="ptt", tag="pt")
                        nc.scalar.activation(pt, sps, Exp, scale=SCALE)
                    for j in range(2):
                        nc.tensor.matmul(
                            cps[0:65, j * 512:(j + 1) * 512],
                            lhsT=vh[kb][:, 65 * h:65 * h + 65],
                            rhs=pt[:, j * 512:(j + 1) * 512],
                            start=(kb == 0), stop=(kb == 15))
                # stage ctx+denominator to SBUF, freeing the PSUM bank;
                # normalize on DVE off the PE critical path
                stg = stp.tile([P, 1024], f32, name="stg", tag="st")
                nc.vector.tensor_copy(stg[0:65, :], cps[0:65, :])
                # half-width broadcast/recip chains to halve SBUF footprint;
                # each half stages into its own [1,512] tile so the raw
                # broadcast AP has tile-base offset 0
                for j in range(2):
                    rc = sm.tile([1, 512], f32, name="rc", tag="rc")
                    nc.gpsimd.dma_start(
                        out=rc, in_=stg[64:65, j * 512:(j + 1) * 512])
                    step = (list(rc.ap[1])[0]
                            if hasattr(rc.ap[1], "__iter__") else 1)
                    bc = sm.tile([64, 512], f32, name="bc", tag="bc")
                    nc.gpsimd.dma_start(
                        out=bc,
                        in_=bass.AP(tensor=rc.tensor, offset=rc.offset,
                                    ap=[[1, 1], [0, 64], [step, 512]]))
                    rb = sm.tile([64, 512], f32, name="rb", tag="rb")
                    nc.vector.reciprocal_approx_fast(rb, bc)
                    nc.vector.tensor_mul(
                        ctxT[2 * p + qh][po:po + 64, j * 512:(j + 1) * 512],
                        stg[0:64, j * 512:(j + 1) * 512], rb)

            # ---------------- output projection ----------------
            def outproj(qh):
                for qbl in range(8):
                    qb = qh * 8 + qbl
                    for n in range(2):
                        oa = pp.tile([P, 512], f32, name="ops", tag="pp")
                        for p in range(4):
                            nc.tensor.matmul(
                                oa,
                                lhsT=ctxT[2 * p + qh][:, qbl * P:(qbl + 1) * P],
                                rhs=wo_sb[:, p, n * 512:(n + 1) * 512],
                                start=(p == 0), stop=(p == 3))
                        ot = otp.tile([P, 512], bf16, name="ot", tag="ot")
                        nc.vector.tensor_copy(ot, oa)
                        [nc.sync, nc.gpsimd][qb % 2].dma_start(
                            out=out_d.ap()[qb * P:(qb + 1) * P,
                                           n * 512:(n + 1) * 512],
                            in_=ot)

            # qh-major: all of query-half 0 first, so its output projection
            # overlaps the second half of attention
            for qh in range(2):
                for p in range(4):
                    for hh in range(2):
                        attn_chunk(p, hh, qh)
                outproj(qh)

    nc.finalize()
    return nc


def _prep_in_maps(q, k, v, Wq, bq, Wk, bk, Wv, bv, Wo, bo):
    in_maps = []
    for c in range(NCORES):
        b, g = c // 2, c % 2
        sl = slice(g * DL, (g + 1) * DL)
        bvl = np.asarray(bv)[sl].astype(np.float32)
        bvb = np.zeros(520, np.float32)
        for h in range(8):
            bvb[65 * h:65 * h + 64] = bvl[64 * h:64 * h + 64]
        bvb = np.broadcast_to(bvb, (P, 520))
        def tile_x(x):
            xt = np.ascontiguousarray(np.asarray(x)[b].T).astype(BF16)  # (1024, 2048)
            return xt.reshape(8, P, S)
        def tile_w_pair(w):
            # (1024, 512) d_in x d_out_local -> (P, 4 pairs, 8 c, 128)
            return np.ascontiguousarray(
                np.asarray(w).reshape(8, P, 4, P).transpose(1, 2, 0, 3))
        def tile_w(w):
            # (1024, DL) -> (P, 8, DL) partition-major
            return np.ascontiguousarray(
                np.asarray(w).reshape(8, P, -1).transpose(1, 0, 2))
        in_maps.append({
            "xq": tile_x(q),
            "xk": tile_x(k),
            "xv": tile_x(v),
            "wq": tile_w_pair(np.asarray(Wq)[sl, :].T.astype(BF16)),
            "wk0": tile_w_pair(np.asarray(Wk)[sl, :].T.astype(BF16))[:, 0],
            "wk123": np.ascontiguousarray(
                tile_w_pair(np.asarray(Wk)[sl, :].T.astype(BF16))[:, 1:4]),
            "wv": tile_w(np.asarray(Wv)[sl, :].T.astype(BF16)),
            "wo": np.ascontiguousarray(
                np.asarray(Wo)[:, sl].T.astype(BF16).reshape(4, P, D).transpose(1, 0, 2)),
            "bq": np.ascontiguousarray(np.asarray(bq)[sl].reshape(4, P).T).astype(np.float32),
            "bk": np.ascontiguousarray(np.asarray(bk)[sl].reshape(4, P).T).astype(np.float32),
            "bvb": np.ascontiguousarray(bvb).astype(BF16),
        })
    return in_maps


def _get_runner():
    """Build nc + jitted SPMD executor once; reuse across kernel() calls."""
    if "runner" in _CACHE:
        return _CACHE["runner"]
    import jax
    import jax.numpy as jnp
    from jax.sharding import Mesh, PartitionSpec
    from jax.experimental.shard_map import shard_map
    from concourse import mybir
    from concourse.bass2jax import (_bass_exec_p, install_neuronx_cc_hook,
                                    partition_id_tensor)

    nc = _build_nc()
    install_neuronx_cc_hook()

    partition_name = nc.partition_id_tensor.name if nc.partition_id_tensor else None
    in_names, out_names, out_avals, zero_shapes = [], [], [], []
    for alloc in nc.m.functions[0].allocations:
        if not isinstance(alloc, mybir.MemoryLocationSet):
            continue
        name = alloc.memorylocations[0].name
        if alloc.kind == "ExternalInput":
            if name != partition_name:
                in_names.append(name)
        elif alloc.kind == "ExternalOutput":
            shape = tuple(alloc.tensor_shape)
            dtype = mybir.dt.np(alloc.dtype)
            out_names.append(name)
            out_avals.append(jax.core.ShapedArray(shape, dtype))
            zero_shapes.append((shape, dtype))
    n_params = len(in_names)
    all_in_names = list(in_names) + list(out_names)
    if partition_name is not None:
        all_in_names.append(partition_name)

    def _body(*args):
        operands = list(args)
        if partition_name is not None:
            operands.append(partition_id_tensor())
        outs = _bass_exec_p.bind(
            *operands,
            out_avals=tuple(out_avals),
            in_names=tuple(all_in_names),
            out_names=tuple(out_names),
            lowering_input_output_aliases=(),
            sim_require_finite=True,
            sim_require_nnan=True,
            nc=nc,
        )
        return tuple(outs)

    devices = jax.devices()[:NCORES]
    mesh = Mesh(np.asarray(devices), ("core",))
    n_outs = len(out_names)
    sharded = jax.jit(
        shard_map(_body, mesh=mesh,
                  in_specs=(PartitionSpec("core"),) * (n_params + n_outs),
                  out_specs=(PartitionSpec("core"),) * n_outs,
                  check_rep=False),
        donate_argnums=tuple(range(n_params, n_params + n_outs)),
        keep_unused=True,
    )
    runner = dict(nc=nc, sharded=sharded, in_names=in_names,
                  out_names=out_names, zero_shapes=zero_shapes,
                  out_avals=out_avals)
    _CACHE["runner"] = runner
    return runner


def kernel(q, k, v, Wq, bq, Wk, bk, Wv, bv, Wo, bo):
    global LAST_RESULTS
    r = _get_runner()
    in_maps = _prep_in_maps(q, k, v, Wq, bq, Wk, bk, Wv, bv, Wo, bo)

    concat_in = [np.concatenate([m[name] for m in in_maps], axis=0)
                 for name in r["in_names"]]
    concat_zeros = [np.zeros((NCORES * s[0], *s[1:]), d)
                    for (s, d) in r["zero_shapes"]]
    out_arrs = r["sharded"](*concat_in, *concat_zeros)
    results = [
        {name: np.asarray(out_arrs[i]).reshape(NCORES, *r["out_avals"][i].shape)[c]
         for i, name in enumerate(r["out_names"])}
        for c in range(NCORES)
    ]
    LAST_RESULTS = results

    bo_f = np.asarray(bo).astype(np.float32)
    out = np.empty((B, S, D), np.float32)
    for b in range(B):
        out[b] = (results[2 * b]["out"].astype(np.float32)
                  + results[2 * b + 1]["out"].astype(np.float32)
                  + bo_f)
    return out


# revision 17
# speedup vs baseline: 1.2647x; 1.2401x over previous
"""MultiHeadAttention Trainium2 kernel (8 NeuronCores, SPMD).

Sharding: core c = (batch b=c//2, head-group g=c%2). Each core computes
8 of the 16 heads for one batch: Q/K/V projections restricted to the
512 d_model dims of its head group, full attention for those heads, and
a partial output projection. Host sums the two head-group partials per
batch and adds the output bias.

Single merged pipeline (v2). The kernel is ACT(exp)-floor-limited at
~278us (256 x [128,1024] exps at 1.0855us each) and PE-streaming-limited
at ~330us (1536 N=512 matmuls). All phases overlap:
  - DMA order: biases, wq, xq, wk(pair0), xk, wv, xv, wk(pairs1-3), wo.
    Exps can start as soon as qhT/khT pair0 exist (~44us, DMA-bound).
  - ALL Q projections run in the DMA-bound preamble (PE idle anyway).
  - scores+exp are emitted with tc.high_priority: the Tile scheduler
    always prefers them, so ACT never waits on PE for its next tile.
  - K pairs 1-3, V projection, ctx matmuls, and the output projection
    fill remaining PE slots by dependency readiness (priority heap).
  - chunks run qh-major so the output projection for query-half 0 can
    overlap the entire second half of attention.
  - xv reuses the xq SBUF tiles (WAR after Q proj, which is done by the
    time xk finishes); wo/wk134 arrive during attention.

PSUM: sps 2x[128,1024] (4 banks) + ctx accum [128,1024] (2) + shared
proj/outproj pool 2x[128,512] (2) = 8 banks exactly.
"""

import numpy as np
import ml_dtypes

BF16 = ml_dtypes.bfloat16

B, S, D, H = 4, 2048, 1024, 16
DH = 64          # head dim
DL = 512         # local d_out (8 heads)
P = 128          # partitions
NCORES = 8
SCALE = 1.0 / np.sqrt(DH)
HIP = 1_000_000  # high-priority offset for scores+exp

_CACHE = {}
LAST_RESULTS = None  # stashed BassKernelResults for test harness


def _build_nc():
    import concourse.bass as bass
    from concourse import bacc, mybir
    from concourse.tile import TileContext

    f32 = mybir.dt.float32
    bf16 = mybir.dt.bfloat16

    nc = bacc.Bacc("TRN2", target_bir_lowering=False, debug=False, num_devices=NCORES)

    # x pre-transposed on host: (8 d_in chunks, 128, 2048 tokens)
    xq_d = nc.dram_tensor("xq", (8, P, S), bf16, kind="ExternalInput")
    xk_d = nc.dram_tensor("xk", (8, P, S), bf16, kind="ExternalInput")
    xv_d = nc.dram_tensor("xv", (8, P, S), bf16, kind="ExternalInput")
    # wq/wk pair-major: (P, 4 pairs, 8 c-chunks, 128) so a pair is one
    # contiguous 2KB/partition DMA
    wq_d = nc.dram_tensor("wq", (P, 4, 8, P), bf16, kind="ExternalInput")
    wk0_d = nc.dram_tensor("wk0", (P, 8, P), bf16, kind="ExternalInput")
    wk123_d = nc.dram_tensor("wk123", (P, 3, 8, P), bf16, kind="ExternalInput")
    wv_d = nc.dram_tensor("wv", (P, 8, DL), bf16, kind="ExternalInput")
    wo_d = nc.dram_tensor("wo", (P, 4, D), bf16, kind="ExternalInput")
    bq_d = nc.dram_tensor("bq", (P, 4), f32, kind="ExternalInput")
    bk_d = nc.dram_tensor("bk", (P, 4), f32, kind="ExternalInput")
    bvb_d = nc.dram_tensor("bvb", (P, 520), bf16, kind="ExternalInput")
    out_d = nc.dram_tensor("out", (S, D), bf16, kind="ExternalOutput")

    Exp = mybir.ActivationFunctionType.Exp

    with TileContext(nc) as tc:
        with tc.tile_pool(name="res", bufs=1) as res, \
             tc.tile_pool(name="pp", bufs=2, space="PSUM") as pp, \
             tc.tile_pool(name="sc", bufs=2, space="PSUM") as sc, \
             tc.tile_pool(name="cx", bufs=1, space="PSUM") as cx, \
             tc.tile_pool(name="pt", bufs=10) as ptp, \
             tc.tile_pool(name="st", bufs=2) as stp, \
             tc.tile_pool(name="sm", bufs=2) as sm, \
             tc.tile_pool(name="ot", bufs=2) as otp:

            # ---------------- resident SBUF ----------------
            wq_sb = res.tile([P, 4, 8, P], bf16)
            wk0_sb = res.tile([P, 8, P], bf16)
            wk123_sb = res.tile([P, 3, 8, P], bf16)
            wv_sb = res.tile([P, 8, DL], bf16)
            wo_sb = res.tile([P, 4, D], bf16)
            bq_sb = res.tile([P, 4], f32)
            bk_sb = res.tile([P, 4], f32)
            bvb_sb = res.tile([P, 520], bf16)

            xa = [res.tile([P, S], bf16, name=f"xa{c}") for c in range(8)]
            xb = [res.tile([P, S], bf16, name=f"xb{c}") for c in range(8)]

            qhT = [res.tile([P, S], bf16, name=f"qhT{i}") for i in range(4)]
            khT = [res.tile([P, S], bf16, name=f"khT{i}") for i in range(4)]
            vh = [res.tile([P, 8 * 65], bf16, name=f"vh{i}") for i in range(16)]
            ctxT = [res.tile([P, 1024], bf16, name=f"ctxT{i}") for i in range(8)]

            # ---------------- DMAs in priority order ----------------
            # scalar (ACT) queue is free until exps start (~44us): use it
            # for xq/xk; xv/wk123/wo overlap the exp phase -> sync/gpsimd
            qs = [nc.sync, nc.scalar, nc.gpsimd]
            q2 = [nc.sync, nc.gpsimd]
            nc.sync.dma_start(out=bq_sb, in_=bq_d.ap())
            nc.scalar.dma_start(out=bk_sb, in_=bk_d.ap())
            nc.gpsimd.dma_start(out=bvb_sb, in_=bvb_d.ap())
            nc.sync.dma_start(out=wq_sb, in_=wq_d.ap())
            for c in range(8):
                qs[c % 3].dma_start(out=xa[c], in_=xq_d.ap()[c])
            nc.sync.dma_start(out=wk0_sb, in_=wk0_d.ap())
            for c in range(8):
                qs[c % 3].dma_start(out=xb[c], in_=xk_d.ap()[c])
            nc.sync.dma_start(out=wv_sb, in_=wv_d.ap())

            # ---------------- projections ----------------
            def proj_qk(x_sb, wsel, b_sb, dst, pairs):
                # x-streaming, one PSUM tile at a time (c-inner): only one
                # accumulation group live -> fits the 2-slot pp pool
                for p in pairs:
                    for t in range(4):
                        ps = pp.tile([P, 512], f32, name="pjt", tag="pp")
                        for c in range(8):
                            nc.tensor.matmul(
                                ps, lhsT=wsel(p, c),
                                rhs=x_sb[c][:, t * 512:(t + 1) * 512],
                                start=(c == 0), stop=(c == 7))
                        nc.vector.tensor_scalar_add(
                            out=dst[p][:, t * 512:(t + 1) * 512],
                            in0=ps, scalar1=b_sb[:, p:p + 1])

            wq_w = lambda p, c: wq_sb[:, p, c, :]
            wk_w = lambda p, c: (wk0_sb[:, c, :] if p == 0
                                 else wk123_sb[:, p - 1, c, :])
            # Q0 then K0 (K0 gates exp start and jumps ahead of Q1-3 as
            # xk chunks land); Q1-3 fill the DMA-bound preamble
            proj_qk(xa, wq_w, bq_sb, qhT, [0])
            proj_qk(xb, wk_w, bk_sb, khT, [0])
            proj_qk(xa, wq_w, bq_sb, qhT, [1, 2, 3])
            # xv reuses the xa tiles: emitted AFTER the Q projection so each
            # chunk's DMA waits (WAR) for Q-proj's reads (Tile deps follow
            # emission order)
            for c in range(8):
                qs[c % 3].dma_start(out=xa[c], in_=xv_d.ap()[c])
            nc.sync.dma_start(out=wk123_sb, in_=wk123_d.ap())
            nc.sync.dma_start(out=wo_sb, in_=wo_d.ap())
            # K1 fills the PE while xv is still in flight and must be
            # done before chunk 2 (pair 1) needs it
            proj_qk(xb, wk_w, bk_sb, khT, [1])

            # V projection: x-stationary so the output lands token-major.
            # Emitted before the attention chunks (and before K1-3) so the
            # scheduler prefers it as soon as xv lands -> vh[kb] feeds ctx.
            for tb in range(16):
                psv = pp.tile([P, 512], f32, name="psv", tag="pp")
                for c in range(8):
                    nc.tensor.matmul(
                        psv, lhsT=xa[c][:, tb * P:(tb + 1) * P],
                        rhs=wv_sb[:, c, :],
                        start=(c == 0), stop=(c == 7))
                vt = vh[tb].rearrange("p (h e) -> p h e", e=65)
                nc.vector.tensor_copy(
                    vt[:, :, 0:64],
                    psv.rearrange("p (h e) -> p h e", e=64))
                nc.gpsimd.memset(vt[:, :, 64:65], 1.0)
                nc.vector.tensor_add(vh[tb], vh[tb], bvb_sb)

            proj_qk(xb, wk_w, bk_sb, khT, [2, 3])

            # ---------------- attention ----------------
            def attn_chunk(p, hh, qh):
                h = 2 * p + hh
                po = 64 * hh
                q0 = qh * 1024
                cps = cx.tile([P, 1024], f32, name="cps", tag="cx")
                pts = []
                for kb in range(16):
                    with tc.high_priority(offset=HIP):
                        sps = sc.tile([P, 1024], f32, name="sps", tag="s")
                        for j in range(2):
                            nc.tensor.matmul(
                                sps[:, j * 512:(j + 1) * 512],
                                lhsT=khT[p][po:po + 64, kb * P:(kb + 1) * P],
                                rhs=qhT[p][po:po + 64, q0 + j * 512:q0 + (j + 1) * 512],
                                start=True, stop=True)
                        pt = ptp.tile([P, 1024], bf16, name="ptt", tag="pt")
                        nc.scalar.activation(pt, sps, Exp, scale=SCALE)
                    for j in range(2):
                        nc.tensor.matmul(
                            cps[0:65, j * 512:(j + 1) * 512],
                            lhsT=vh[kb][:, 65 * h:65 * h + 65],
                            rhs=pt[:, j * 512:(j + 1) * 512],
                            start=(kb == 0), stop=(kb == 15))
                # stage ctx+denominator to SBUF, freeing the PSUM bank;
                # normalize on DVE off the PE critical path
                stg = stp.tile([P, 1024], f32, name="stg", tag="st")
                nc.vector.tensor_copy(stg[0:65, :], cps[0:65, :])
                # denominator row -> partition 0 (one small DMA), then
                # Pool-engine broadcast (no second DMA hop)
                for j in range(2):
                    rc = sm.tile([1, 512], f32, name="rc", tag="rc")
                    nc.gpsimd.dma_start(
                        out=rc, in_=stg[64:65, j * 512:(j + 1) * 512])
                    bc = sm.tile([64, 512], f32, name="bc", tag="bc")
                    nc.gpsimd.partition_broadcast(bc, rc)
                    rb = sm.tile([64, 512], f32, name="rb", tag="rb")
                    nc.vector.reciprocal_approx_fast(rb, bc)
                    nc.vector.tensor_mul(
                        ctxT[2 * p + qh][po:po + 64, j * 512:(j + 1) * 512],
                        stg[0:64, j * 512:(j + 1) * 512], rb)

            # ---------------- output projection ----------------
            def outproj(qh):
                for qbl in range(8):
                    qb = qh * 8 + qbl
                    for n in range(2):
                        oa = pp.tile([P, 512], f32, name="ops", tag="pp")
                        for p in range(4):
                            nc.tensor.matmul(
                                oa,
                                lhsT=ctxT[2 * p + qh][:, qbl * P:(qbl + 1) * P],
                                rhs=wo_sb[:, p, n * 512:(n + 1) * 512],
                                start=(p == 0), stop=(p == 3))
                        ot = otp.tile([P, 512], bf16, name="ot", tag="ot")
                        nc.vector.tensor_copy(ot, oa)
                        nc.sync.dma_start(
                            out=out_d.ap()[qb * P:(qb + 1) * P,
                                           n * 512:(n + 1) * 512],
                            in_=ot)

            # qh-major: all of query-half 0 first, so its output projection
            # overlaps the second half of attention
            for qh in range(2):
                for p in range(4):
                    for hh in range(2):
                        attn_chunk(p, hh, qh)
                outproj(qh)

    nc.finalize()
    return nc


def _prep_in_maps(q, k, v, Wq, bq, Wk, bk, Wv, bv, Wo, bo):
    in_maps = []
    for c in range(NCORES):
        b, g = c // 2, c % 2
        sl = slice(g * DL, (g + 1) * DL)
        bvl = np.asarray(bv)[sl].astype(np.float32)
        bvb = np.zeros(520, np.float32)
        for h in range(8):
            bvb[65 * h:65 * h + 64] = bvl[64 * h:64 * h + 64]
        bvb = np.broadcast_to(bvb, (P, 520))
        def tile_x(x):
            xt = np.ascontiguousarray(np.asarray(x)[b].T).astype(BF16)  # (1024, 2048)
            return xt.reshape(8, P, S)
        def tile_w_pair(w):
            # (1024, 512) d_in x d_out_local -> (P, 4 pairs, 8 c, 128)
            return np.ascontiguousarray(
                np.asarray(w).reshape(8, P, 4, P).transpose(1, 2, 0, 3))
        def tile_w(w):
            # (1024, DL) -> (P, 8, DL) partition-major
            return np.ascontiguousarray(
                np.asarray(w).reshape(8, P, -1).transpose(1, 0, 2))
        in_maps.append({
            "xq": tile_x(q),
            "xk": tile_x(k),
            "xv": tile_x(v),
            "wq": tile_w_pair(np.asarray(Wq)[sl, :].T.astype(BF16)),
            "wk0": tile_w_pair(np.asarray(Wk)[sl, :].T.astype(BF16))[:, 0],
            "wk123": np.ascontiguousarray(
                tile_w_pair(np.asarray(Wk)[sl, :].T.astype(BF16))[:, 1:4]),
            "wv": tile_w(np.asarray(Wv)[sl, :].T.astype(BF16)),
            "wo": np.ascontiguousarray(
                np.asarray(Wo)[:, sl].T.astype(BF16).reshape(4, P, D).transpose(1, 0, 2)),
            "bq": np.ascontiguousarray(np.asarray(bq)[sl].reshape(4, P).T).astype(np.float32),
            "bk": np.ascontiguousarray(np.asarray(bk)[sl].reshape(4, P).T).astype(np.float32),
            "bvb": np.ascontiguousarray(bvb).astype(BF16),
        })
    return in_maps


def _get_runner():
    """Build nc + jitted SPMD executor once; reuse across kernel() calls."""
    if "runner" in _CACHE:
        return _CACHE["runner"]
    import jax
    import jax.numpy as jnp
    from jax.sharding import Mesh, PartitionSpec
    from jax.experimental.shard_map import shard_map
    from concourse import mybir
    from concourse.bass2jax import (_bass_exec_p, install_neuronx_cc_hook,
                                    partition_id_tensor)

    nc = _build_nc()
    install_neuronx_cc_hook()

    partition_name = nc.partition_id_tensor.name if nc.partition_id_tensor else None
    in_names, out_names, out_avals, zero_shapes = [], [], [], []
    for alloc in nc.m.functions[0].allocations:
        if not isinstance(alloc, mybir.MemoryLocationSet):
            continue
        name = alloc.memorylocations[0].name
        if alloc.kind == "ExternalInput":
            if name != partition_name:
                in_names.append(name)
        elif alloc.kind == "ExternalOutput":
            shape = tuple(alloc.tensor_shape)
            dtype = mybir.dt.np(alloc.dtype)
            out_names.append(name)
            out_avals.append(jax.core.ShapedArray(shape, dtype))
            zero_shapes.append((shape, dtype))
    n_params = len(in_names)
    all_in_names = list(in_names) + list(out_names)
    if partition_name is not None:
        all_in_names.append(partition_name)

    def _body(*args):
        operands = list(args)
        if partition_name is not None:
            operands.append(partition_id_tensor())
        outs = _bass_exec_p.bind(
            *operands,
            out_avals=tuple(out_avals),
            in_names=tuple(all_in_names),
            out_names=tuple(out_names),
            lowering_input_output_aliases=(),
            sim_require_finite=True,
            sim_require_nnan=True,
            nc=nc,
        )
        return tuple(outs)

    devices = jax.devices()[:NCORES]
    mesh = Mesh(np.asarray(devices), ("core",))
    n_outs = len(out_names)
    sharded = jax.jit(
        shard_map(_body, mesh=mesh,
                  in_specs=(PartitionSpec("core"),) * (n_params + n_outs),
                  out_specs=(PartitionSpec("core"),) * n_outs,
                  check_rep=False),
        donate_argnums=tuple(range(n_params, n_params + n_outs)),
        keep_unused=True,
    )
    runner = dict(nc=nc, sharded=sharded, in_names=in_names,
                  out_names=out_names, zero_shapes=zero_shapes,
                  out_avals=out_avals)
    _CACHE["runner"] = runner
    return runner


def kernel(q, k, v, Wq, bq, Wk, bk, Wv, bv, Wo, bo):
    global LAST_RESULTS
    r = _get_runner()
    in_maps = _prep_in_maps(q, k, v, Wq, bq, Wk, bk, Wv, bv, Wo, bo)

    concat_in = [np.concatenate([m[name] for m in in_maps], axis=0)
                 for name in r["in_names"]]
    concat_zeros = [np.zeros((NCORES * s[0], *s[1:]), d)
                    for (s, d) in r["zero_shapes"]]
    out_arrs = r["sharded"](*concat_in, *concat_zeros)
    results = [
        {name: np.asarray(out_arrs[i]).reshape(NCORES, *r["out_avals"][i].shape)[c]
         for i, name in enumerate(r["out_names"])}
        for c in range(NCORES)
    ]
    LAST_RESULTS = results

    bo_f = np.asarray(bo).astype(np.float32)
    out = np.empty((B, S, D), np.float32)
    for b in range(B):
        out[b] = (results[2 * b]["out"].astype(np.float32)
                  + results[2 * b + 1]["out"].astype(np.float32)
                  + bo_f)
    return out
